# revision 13
# baseline (speedup 1.0000x reference)
"""Trainium2 Bass kernel for multi-head causal attention with RoPE.

Problem: x[2,2048,1024] @ {qw,kw,vw}[1024,1024] -> 16-head causal attention
with interleaved RoPE on Q,K -> @ ow[1024,1024].

Sharding (8 cores): core c handles batch b=c//4, head-group g=c%4 (4 heads).
qw/kw/vw column-sharded, ow row-sharded (Megatron attention parallelism);
the host sums the 4 partial outputs per batch.

Device-side layout strategy (per core):
- Host feeds x transposed (xT [1024,2048]) and qw/kw columns permuted so each
  head's RoPE pairs are deinterleaved ([evens|odds]); RoPE becomes contiguous
  free-dim ops. The d-permutation cancels in Q.K^T dot products.
- Q,K computed as [s,d] tiles, RoPE'd, PE-transposed to QT/KT [d,s] stored
  head-pair-stacked ([128, 2048] per pair, head at partitions 0-63/64-127).
- Attention computed transposed: S^T[k,q] = KT^T @ QT per 128-k-block.
  Scores are tiny (|S|<~1) so softmax needs no max subtraction: W^T =
  exp(S^T/8) evicted straight from PSUM by the scalar engine into f32r tiles.
  Causal mask = upper-tri multiply on diagonal 128x128 blocks only; blocks
  right of the diagonal are never computed.
- PV: stationary = [V_h | ones] (M=65) so row 64 of the PSUM accumulator is
  the softmax denominator for free.
- attnT[65,512] -> PE-transpose -> [q,65]: reciprocal of col 64, per-partition
  (per-q) scale of cols 0-63, PE-transpose back to [64,q], DMA
  (partition-shifting) into pair-stacked mergedT tiles -> out-projection.
- All matmuls run in float32r (full PE rate at N>=256, ~tf32 precision).
"""

import sys

sys.path.insert(0, "/opt/trn_rl_repo")

import numpy as np
import ml_dtypes

BF = ml_dtypes.bfloat16
from contextlib import ExitStack

import concourse.bass as bass
import concourse.bacc as bacc
import concourse.tile as tile
from concourse import mybir
from concourse.bass_utils import run_bass_kernel_spmd

F32 = mybir.dt.float32
F32R = mybir.dt.float32r
BF16 = mybir.dt.bfloat16
Exp = mybir.ActivationFunctionType.Exp
MULT = mybir.AluOpType.mult

D_MODEL, N_HEADS, BATCH, SEQ = 1024, 16, 2, 2048
HEAD_DIM = 64
N_CORES = 8
GH = 4  # heads per core
GD = GH * HEAD_DIM  # 256 cols per core
NB = SEQ // 128  # 16 s-blocks
NQC = SEQ // 512  # 4 q-chunks


def build_program(phases=(1, 2, 3)):
    nc = bacc.Bacc("TRN2", target_bir_lowering=False)

    xT = nc.declare_dram_parameter("xT", [D_MODEL, SEQ], BF16, isOutput=False)
    wq = nc.declare_dram_parameter("wq", [D_MODEL, GD], BF16, isOutput=False)
    wk = nc.declare_dram_parameter("wk", [D_MODEL, GD], BF16, isOutput=False)
    wv = nc.declare_dram_parameter("wv", [D_MODEL, GD], BF16, isOutput=False)
    wo = nc.declare_dram_parameter("wo", [GD, D_MODEL], BF16, isOutput=False)
    cos4 = nc.declare_dram_parameter("cos4", [SEQ, GD], BF16, isOutput=False)
    sin4 = nc.declare_dram_parameter("sin4", [SEQ, GD], BF16, isOutput=False)
    tri = nc.declare_dram_parameter("tri", [128, 128], BF16, isOutput=False)
    idn = nc.declare_dram_parameter("idn", [128, 128], BF16, isOutput=False)
    idn32 = nc.declare_dram_parameter("idn32", [128, 128], F32, isOutput=False)
    ones = nc.declare_dram_parameter("ones", [128, NB * GH], BF16, isOutput=False)
    out = nc.declare_dram_parameter("out", [SEQ, D_MODEL], F32, isOutput=True)

    with tile.TileContext(nc) as tc, ExitStack() as ctx:
        res = ctx.enter_context(tc.tile_pool(name="res", bufs=1))

        wq_sb = res.tile([128, 8 * GD], BF16, tag="wq")
        wk_sb = res.tile([128, 8 * GD], BF16, tag="wk")
        wv_sb = res.tile([128, 8 * GD], BF16, tag="wv")
        wo_sb = res.tile([128, 2 * D_MODEL], BF16, tag="wo")
        cos_sb = res.tile([128, NB * GD], BF16, tag="cos")
        sin_sb = res.tile([128, NB * GD], BF16, tag="sin")
        tri_sb = res.tile([128, 128], BF16, tag="tri")
        idn_sb = res.tile([128, 128], BF16, tag="idn")
        idn32_sb = res.tile([128, 128], F32, tag="idn32")
        qt_sb = [res.tile([128, SEQ], BF16, tag=f"qt{hp}", name=f"qt{hp}") for hp in range(2)]
        kt_sb = [res.tile([128, SEQ], BF16, tag=f"kt{hp}", name=f"kt{hp}") for hp in range(2)]
        vaug_sb = res.tile([128, NB * GH * 65], BF16, tag="vaug")
        mt_sb = res.tile([128, NB * 2 * 128], BF16, tag="mt")

        for c in range(8):
            nc.sync.dma_start(
                wq_sb[:].rearrange("p (c d) -> p c d", c=8)[:, c : c + 1, :],
                wq[:].rearrange("(c p) d -> p c d", p=128)[:, c : c + 1, :],
            )
        for c in range(8):
            nc.sync.dma_start(
                wk_sb[:].rearrange("p (c d) -> p c d", c=8)[:, c : c + 1, :],
                wk[:].rearrange("(c p) d -> p c d", p=128)[:, c : c + 1, :],
            )
        for c in range(8):
            nc.sync.dma_start(
                wv_sb[:].rearrange("p (c d) -> p c d", c=8)[:, c : c + 1, :],
                wv[:].rearrange("(c p) d -> p c d", p=128)[:, c : c + 1, :],
            )
        nc.sync.dma_start(
            wo_sb[:].rearrange("p (c d) -> p c d", c=2),
            wo[:].rearrange("(c p) d -> p c d", p=128),
        )
        nc.sync.dma_start(
            cos_sb[:].rearrange("p (n d) -> p n d", n=NB),
            cos4[:].rearrange("(n p) d -> p n d", p=128),
        )
        nc.sync.dma_start(
            sin_sb[:].rearrange("p (n d) -> p n d", n=NB),
            sin4[:].rearrange("(n p) d -> p n d", p=128),
        )
        nc.sync.dma_start(tri_sb[:], tri[:])
        nc.sync.dma_start(idn_sb[:], idn[:])
        nc.sync.dma_start(idn32_sb[:], idn32[:])

        # ones columns of the V-augmented tiles (col 64 of each 65-block)
        nc.sync.dma_start(
            vaug_sb[:].rearrange("p (n h e) -> p n h e", n=NB, h=GH)[:, :, :, 64:65],
            ones[:].rearrange("p (n h) -> p n h", n=NB).unsqueeze(3),
        )

        # warm the exp table load during phase 1
        with tc.tile_pool(name="warm", bufs=1) as wpool:
            scratch = wpool.tile([128, 1], F32)
            nc.vector.memset(scratch[:], 0.0)
            nc.scalar.activation(scratch[:], scratch[:], Exp)

        xT_r = xT[:].rearrange("(c p) s -> p c s", p=128)
        cos_v = cos_sb[:].rearrange("p (n d) -> p n d", n=NB)
        sin_v = sin_sb[:].rearrange("p (n d) -> p n d", n=NB)
        vaug_v = vaug_sb[:].rearrange("p (n h e) -> p n h e", n=NB, h=GH)
        mt_v = mt_sb[:].rearrange("p (n t q) -> p n t q", n=NB, t=2)

        # ---------------- Phase 1: QKV projections + RoPE + transposes -----
        if 1 not in phases:
            phases = phases
        with (
            tc.tile_pool(name="p1x", bufs=3) as sp_x,
            tc.tile_pool(name="p1t", bufs=6) as sp_t,
            tc.tile_pool(name="p1ps", bufs=2, space="PSUM") as pp_proj,
            tc.tile_pool(name="p1tr", bufs=2, space="PSUM") as pp_tr,
        ):
            for i in (range(NB) if 1 in phases else []):
                xts = sp_x.tile([128, 8 * 128], BF16, tag="xts")
                nc.sync.dma_start(
                    xts[:].rearrange("p (c s) -> p c s", c=8),
                    xT_r[:, :, i * 128 : (i + 1) * 128],
                )
                xts_v = xts[:].rearrange("p (c s) -> p c s", c=8)

                for wsb, kind in ((wq_sb, "q"), (wk_sb, "k"), (wv_sb, "v")):
                    ps = pp_proj.tile([128, GD], F32, tag="proj")
                    wv_ = wsb[:].rearrange("p (c d) -> p c d", c=8)
                    for c in range(8):
                        nc.tensor.matmul(
                            ps[:],
                            xts_v[:, c, :],
                            wv_[:, c, :],
                            start=(c == 0),
                            stop=(c == 7),
                        )
                    if kind == "v":
                        # evict V heads into vaug (cols 0-63 of each 65-block)
                        nc.vector.tensor_copy(
                            vaug_v[:, i, :, 0:64],
                            ps[:].rearrange("p (h d) -> p h d", h=GH),
                        )
                    else:
                        # RoPE: rot = P*cos + swap(P)*sin  (halves pre-deinterleaved)
                        pse = sp_t.tile([128, GD], BF16, tag="pse")
                        nc.vector.tensor_copy(pse[:], ps[:])
                        pse_sw = pse[:].rearrange("p (h x d) -> p h x d", h=GH, x=2)[
                            :, :, ::-1, :
                        ]
                        tmp1 = sp_t.tile([128, GD], BF16, tag="tmp1")
                        tmp2 = sp_t.tile([128, GD], BF16, tag="tmp2")
                        rot = sp_t.tile([128, GD], BF16, tag="rot" + kind)
                        nc.vector.tensor_tensor(
                            tmp1[:], pse[:], cos_v[:, i, :], op=MULT
                        )
                        nc.vector.tensor_tensor(
                            tmp2[:].rearrange("p (h x d) -> p h x d", h=GH, x=2),
                            pse_sw,
                            sin_v[:, i, :].rearrange("p (h x d) -> p h x d", h=GH, x=2),
                            op=MULT,
                        )
                        nc.vector.tensor_add(rot[:], tmp1[:], tmp2[:])
                        dst = qt_sb if kind == "q" else kt_sb
                        for hh in range(2):
                            pst = pp_tr.tile([128, 128], BF16, tag="tr")
                            nc.tensor.transpose(
                                pst[:], rot[:, hh * 128 : (hh + 1) * 128], idn_sb[:]
                            )
                            nc.vector.tensor_copy(
                                dst[hh][:, i * 128 : (i + 1) * 128], pst[:]
                            )

        # ---------------- Phase 2: attention ------------------------------
        with (
            tc.tile_pool(name="p2wt", bufs=4) as sp_wt,
            tc.tile_pool(name="p2a", bufs=3) as sp_a,
            tc.tile_pool(name="p2rc", bufs=2) as sp_rc,
            tc.tile_pool(name="p2an", bufs=3) as sp_an,
            tc.tile_pool(name="p2st", bufs=4) as sp_st,
            tc.tile_pool(name="p2s", bufs=2, space="PSUM") as pp_s,
            tc.tile_pool(name="p2pv", bufs=1, space="PSUM") as pp_pv,
            tc.tile_pool(name="p2t", bufs=1, space="PSUM") as pp_t,
            tc.tile_pool(name="p2bt", bufs=1, space="PSUM") as pp_bt,
        ):
            for hp in (range(2) if 2 in phases else []):
                for qc in range(NQC):
                    nk = 4 * qc + 4  # k-blocks (<= diagonal)
                    pv = [
                        pp_pv.tile([65, 512], F32, tag=f"pv{e}", name=f"pv{e}") for e in range(2)
                    ]
                    prev = None
                    for ki in range(nk):
                        j = ki - 4 * qc  # >=0 only for diagonal blocks
                        diag = j >= 0
                        off = j * 128 if diag else 0
                        n = 512 - off
                        ps_s = pp_s.tile([128, 1024], F32, tag="s")
                        wt = sp_wt.tile([128, 1024], BF16, tag="wt")
                        for e in range(2):  # head within pair
                            po = 64 * e
                            nc.tensor.matmul(
                                ps_s[:, e * 512 : e * 512 + n],
                                kt_sb[hp][po : po + 64, ki * 128 : (ki + 1) * 128],
                                qt_sb[hp][
                                    po : po + 64, qc * 512 + off : (qc + 1) * 512
                                ],
                                start=True,
                                stop=True,
                            )
                        ps_v = ps_s[:].rearrange("p (e q) -> p e q", e=2)
                        wt_v = wt[:].rearrange("p (e q) -> p e q", e=2)
                        nc.scalar.activation(
                            wt_v[:, :, 0:n], ps_v[:, :, 0:n], Exp, scale=0.125
                        )
                        if diag:
                            nc.vector.tensor_tensor(
                                wt_v[:, :, 0:128],
                                wt_v[:, :, 0:128],
                                tri_sb[:].unsqueeze(1).broadcast_to([128, 2, 128]),
                                op=MULT,
                            )
                        cur = [(ki, off, n, wt), (ki, off, n, wt)]
                        if prev is not None:
                            for e in range(2):
                                pki, poff, pn, pwt = prev[e]
                                nc.tensor.matmul(
                                    pv[e][0:65, poff:512],
                                    vaug_v[:, pki, 2 * hp + e, :],
                                    pwt[:, e * 512 : e * 512 + pn],
                                    start=(pki == 0),
                                    stop=False,
                                )
                        prev = cur
                    for e in range(2):
                        pki, poff, pn, pwt = prev[e]
                        nc.tensor.matmul(
                            pv[e][0:65, poff:512],
                            vaug_v[:, pki, 2 * hp + e, :],
                            pwt[:, e * 512 : e * 512 + pn],
                            start=(pki == 0),
                            stop=True,
                        )
                    for e in range(2):
                        h = 2 * hp + e
                        a = sp_a.tile([65, 512], F32, tag="a")
                        nc.vector.tensor_copy(a[:], pv[e][0:65, :])
                        ps_t = pp_t.tile([128, 260], F32, tag="t")
                        for j in range(4):
                            nc.tensor.transpose(
                                ps_t[:, j * 65 : (j + 1) * 65],
                                a[0:65, j * 128 : (j + 1) * 128],
                                idn32_sb[0:65, 0:65],
                            )
                        ps_t_v = ps_t[:].rearrange("p (j e) -> p j e", j=4)
                        rc = sp_rc.tile([128, 4], F32, tag="rc")
                        nc.vector.reciprocal(
                            rc[:].rearrange("p (j e) -> p j e", e=1),
                            ps_t_v[:, :, 64:65],
                        )
                        an = sp_an.tile([128, 256], BF16, tag="an")
                        for j in range(4):
                            nc.vector.tensor_scalar(
                                an[:, j * 64 : (j + 1) * 64],
                                ps_t_v[:, j, 0:64],
                                rc[:, j : j + 1],
                                None,
                                op0=MULT,
                            )
                        for j in range(4):
                            qb = qc * 4 + j
                            ps_bt = pp_bt.tile([128, 128], BF16, tag="bt")
                            nc.tensor.transpose(
                                ps_bt[0:64, :],
                                an[:, j * 64 : (j + 1) * 64],
                                idn_sb[:],
                            )
                            if e == 0:
                                nc.vector.tensor_copy(
                                    mt_v[0:64, qb, hp, :], ps_bt[0:64, :]
                                )
                            else:
                                stg = sp_st.tile([64, 128], BF16, tag="stg")
                                nc.vector.tensor_copy(stg[:], ps_bt[0:64, :])
                                nc.sync.dma_start(
                                    mt_v[64:128, qb, hp, :], stg[:]
                                )

        # ---------------- Phase 3: output projection ----------------------
        with (
            tc.tile_pool(name="p3o", bufs=3) as sp_o,
            tc.tile_pool(name="p3ps", bufs=2, space="PSUM") as pp_o,
        ):
            wo_v = wo_sb[:].rearrange("p (c d) -> p c d", c=2)
            for qb in (range(NB) if 3 in phases else []):
                ob = sp_o.tile([128, D_MODEL], F32, tag="ob")
                for oc in range(2):
                    ps_o = pp_o.tile([128, 512], F32, tag="o")
                    for hp in range(2):
                        nc.tensor.matmul(
                            ps_o[:],
                            mt_v[:, qb, hp, :],
                            wo_v[:, hp, oc * 512 : (oc + 1) * 512],
                            start=(hp == 0),
                            stop=(hp == 1),
                        )
                    if oc == 0:
                        nc.vector.tensor_copy(
                            ob[:, oc * 512 : (oc + 1) * 512], ps_o[:]
                        )
                    else:
                        nc.scalar.copy(ob[:, oc * 512 : (oc + 1) * 512], ps_o[:])
                nc.sync.dma_start(out[qb * 128 : (qb + 1) * 128, :], ob[:])

    nc.compile()
    return nc


_NC = None


def _host_tables():
    inv_freq = 1.0 / (10000.0 ** (np.arange(0, HEAD_DIM, 2, dtype=np.float32) / HEAD_DIM))
    pos = np.arange(SEQ, dtype=np.float32)
    freq = pos[:, None] * inv_freq[None, :]  # [SEQ, 32]
    cos = np.cos(freq).astype(np.float32)
    sin = np.sin(freq).astype(np.float32)
    cos4 = np.tile(np.concatenate([cos, cos], axis=1), (1, GH))  # [SEQ, 256]
    sin4 = np.tile(np.concatenate([-sin, sin], axis=1), (1, GH))
    tri = np.triu(np.ones((128, 128), dtype=np.float32))  # keep k<=q
    idn = np.eye(128, dtype=np.float32)
    perm = np.concatenate(
        [h * 64 + np.concatenate([np.arange(0, 64, 2), np.arange(1, 64, 2)]) for h in range(GH)]
    )
    return cos4, sin4, tri, idn, perm


def _in_maps(x, qw, kw, vw, ow):
    cos4, sin4, tri, idn, perm = _host_tables()
    maps = []
    for c in range(N_CORES):
        b, g = c // GH, c % GH
        sl = slice(g * GD, (g + 1) * GD)
        maps.append(
            dict(
                xT=np.ascontiguousarray(x[b].T).astype(BF),
                wq=np.ascontiguousarray(qw[:, sl][:, perm]).astype(BF),
                wk=np.ascontiguousarray(kw[:, sl][:, perm]).astype(BF),
                wv=np.ascontiguousarray(vw[:, sl]).astype(BF),
                wo=np.ascontiguousarray(ow[sl, :]).astype(BF),
                ones=np.ones((128, NB * GH), dtype=BF),
                cos4=cos4.astype(BF),
                sin4=sin4.astype(BF),
                tri=tri.astype(BF),
                idn=idn.astype(BF),
                idn32=idn,
            )
        )
    return maps


def _run(x, qw, kw, vw, ow, trace=False):
    global _NC
    if _NC is None:
        _NC = build_program()
    maps = _in_maps(
        np.asarray(x, dtype=np.float32),
        np.asarray(qw, dtype=np.float32),
        np.asarray(kw, dtype=np.float32),
        np.asarray(vw, dtype=np.float32),
        np.asarray(ow, dtype=np.float32),
    )
    br = run_bass_kernel_spmd(_NC, maps, list(range(N_CORES)), trace=trace)
    out = np.zeros((BATCH, SEQ, D_MODEL), dtype=np.float32)
    for c in range(N_CORES):
        out[c // GH] += br.results[c]["out"]
    return out, br


def kernel(x, qw, kw, vw, ow):
    out, _ = _run(x, qw, kw, vw, ow)
    return out


# revision 14
# speedup vs baseline: 1.0261x; 1.0261x over previous
"""Trainium2 Bass kernel for multi-head causal attention with RoPE.

Problem: x[2,2048,1024] @ {qw,kw,vw}[1024,1024] -> 16-head causal attention
with interleaved RoPE on Q,K -> @ ow[1024,1024].

Sharding (8 cores): core c handles batch b=c//4, head-group g=c%4 (4 heads).
qw/kw/vw column-sharded, ow row-sharded (Megatron attention parallelism);
the host sums the 4 partial outputs per batch.

Device-side layout strategy (per core):
- Host feeds x transposed (xT [1024,2048]) and qw/kw columns permuted so each
  head's RoPE pairs are deinterleaved ([evens|odds]); RoPE becomes contiguous
  free-dim ops. The d-permutation cancels in Q.K^T dot products.
- Q,K computed as [s,d] tiles, RoPE'd, PE-transposed to QT/KT [d,s] stored
  head-pair-stacked ([128, 2048] per pair, head at partitions 0-63/64-127).
- Attention computed transposed: S^T[k,q] = KT^T @ QT per 128-k-block.
  Scores are tiny (|S|<~1) so softmax needs no max subtraction: W^T =
  exp(S^T/8) evicted straight from PSUM by the scalar engine into f32r tiles.
  Causal mask = upper-tri multiply on diagonal 128x128 blocks only; blocks
  right of the diagonal are never computed.
- PV: stationary = [V_h | ones] (M=65) so row 64 of the PSUM accumulator is
  the softmax denominator for free.
- attnT[65,512] -> PE-transpose -> [q,65]: reciprocal of col 64, per-partition
  (per-q) scale of cols 0-63, PE-transpose back to [64,q], DMA
  (partition-shifting) into pair-stacked mergedT tiles -> out-projection.
- All matmuls run in float32r (full PE rate at N>=256, ~tf32 precision).
"""

import sys

sys.path.insert(0, "/opt/trn_rl_repo")

import numpy as np
import ml_dtypes

BF = ml_dtypes.bfloat16
from contextlib import ExitStack

import concourse.bass as bass
import concourse.bacc as bacc
import concourse.tile as tile
from concourse import mybir
from concourse.bass_utils import run_bass_kernel_spmd

F32 = mybir.dt.float32
F32R = mybir.dt.float32r
BF16 = mybir.dt.bfloat16
Exp = mybir.ActivationFunctionType.Exp
MULT = mybir.AluOpType.mult

D_MODEL, N_HEADS, BATCH, SEQ = 1024, 16, 2, 2048
HEAD_DIM = 64
N_CORES = 8
GH = 4  # heads per core
GD = GH * HEAD_DIM  # 256 cols per core
NB = SEQ // 128  # 16 s-blocks
NQC = SEQ // 512  # 4 q-chunks


def build_program(phases=(1, 2, 3)):
    nc = bacc.Bacc("TRN2", target_bir_lowering=False)

    xT = nc.declare_dram_parameter("xT", [D_MODEL, SEQ], BF16, isOutput=False)
    wq = nc.declare_dram_parameter("wq", [D_MODEL, GD], BF16, isOutput=False)
    wk = nc.declare_dram_parameter("wk", [D_MODEL, GD], BF16, isOutput=False)
    wv = nc.declare_dram_parameter("wv", [D_MODEL, GD], BF16, isOutput=False)
    wo = nc.declare_dram_parameter("wo", [GD, D_MODEL], BF16, isOutput=False)
    cos4 = nc.declare_dram_parameter("cos4", [SEQ, GD], BF16, isOutput=False)
    sin4 = nc.declare_dram_parameter("sin4", [SEQ, GD], BF16, isOutput=False)
    tri = nc.declare_dram_parameter("tri", [128, 128], BF16, isOutput=False)
    idn = nc.declare_dram_parameter("idn", [128, 128], BF16, isOutput=False)
    idn32 = nc.declare_dram_parameter("idn32", [128, 128], F32, isOutput=False)
    ones = nc.declare_dram_parameter("ones", [128, NB * GH], BF16, isOutput=False)
    out = nc.declare_dram_parameter("out", [SEQ, D_MODEL], F32, isOutput=True)

    with tile.TileContext(nc) as tc, ExitStack() as ctx:
        res = ctx.enter_context(tc.tile_pool(name="res", bufs=1))

        wq_sb = res.tile([128, 8 * GD], BF16, tag="wq")
        wk_sb = res.tile([128, 8 * GD], BF16, tag="wk")
        wv_sb = res.tile([128, 8 * GD], BF16, tag="wv")
        wo_sb = res.tile([128, 2 * D_MODEL], BF16, tag="wo")
        cos_sb = res.tile([128, NB * GD], BF16, tag="cos")
        sin_sb = res.tile([128, NB * GD], BF16, tag="sin")
        tri_sb = res.tile([128, 128], BF16, tag="tri")
        idn_sb = res.tile([128, 128], BF16, tag="idn")
        idn32_sb = res.tile([128, 128], F32, tag="idn32")
        qt_sb = [res.tile([128, SEQ], BF16, tag=f"qt{hp}", name=f"qt{hp}") for hp in range(2)]
        kt_sb = [res.tile([128, SEQ], BF16, tag=f"kt{hp}", name=f"kt{hp}") for hp in range(2)]
        vaug_sb = res.tile([128, NB * GH * 65], BF16, tag="vaug")
        mt_sb = res.tile([128, NB * 2 * 128], BF16, tag="mt")

        for c in range(8):
            nc.sync.dma_start(
                wq_sb[:].rearrange("p (c d) -> p c d", c=8)[:, c : c + 1, :],
                wq[:].rearrange("(c p) d -> p c d", p=128)[:, c : c + 1, :],
            )
        for c in range(8):
            nc.sync.dma_start(
                wk_sb[:].rearrange("p (c d) -> p c d", c=8)[:, c : c + 1, :],
                wk[:].rearrange("(c p) d -> p c d", p=128)[:, c : c + 1, :],
            )
        for c in range(8):
            nc.sync.dma_start(
                wv_sb[:].rearrange("p (c d) -> p c d", c=8)[:, c : c + 1, :],
                wv[:].rearrange("(c p) d -> p c d", p=128)[:, c : c + 1, :],
            )
        nc.sync.dma_start(
            wo_sb[:].rearrange("p (c d) -> p c d", c=2),
            wo[:].rearrange("(c p) d -> p c d", p=128),
        )
        nc.sync.dma_start(
            cos_sb[:].rearrange("p (n d) -> p n d", n=NB),
            cos4[:].rearrange("(n p) d -> p n d", p=128),
        )
        nc.sync.dma_start(
            sin_sb[:].rearrange("p (n d) -> p n d", n=NB),
            sin4[:].rearrange("(n p) d -> p n d", p=128),
        )
        nc.sync.dma_start(tri_sb[:], tri[:])
        nc.sync.dma_start(idn_sb[:], idn[:])
        nc.sync.dma_start(idn32_sb[:], idn32[:])

        # ones columns of the V-augmented tiles (col 64 of each 65-block)
        nc.sync.dma_start(
            vaug_sb[:].rearrange("p (n h e) -> p n h e", n=NB, h=GH)[:, :, :, 64:65],
            ones[:].rearrange("p (n h) -> p n h", n=NB).unsqueeze(3),
        )

        # warm the exp table load during phase 1
        with tc.tile_pool(name="warm", bufs=1) as wpool:
            scratch = wpool.tile([128, 1], F32)
            nc.vector.memset(scratch[:], 0.0)
            nc.scalar.activation(scratch[:], scratch[:], Exp)

        xT_r = xT[:].rearrange("(c p) s -> p c s", p=128)
        cos_v = cos_sb[:].rearrange("p (n d) -> p n d", n=NB)
        sin_v = sin_sb[:].rearrange("p (n d) -> p n d", n=NB)
        vaug_v = vaug_sb[:].rearrange("p (n h e) -> p n h e", n=NB, h=GH)
        mt_v = mt_sb[:].rearrange("p (n t q) -> p n t q", n=NB, t=2)

        # ---------------- Phase 1: QKV projections + RoPE + transposes -----
        if 1 not in phases:
            phases = phases
        with (
            tc.tile_pool(name="p1x", bufs=3) as sp_x,
            tc.tile_pool(name="p1t", bufs=6) as sp_t,
            tc.tile_pool(name="p1ps", bufs=2, space="PSUM") as pp_proj,
            tc.tile_pool(name="p1tr", bufs=2, space="PSUM") as pp_tr,
        ):
            for i in (range(NB) if 1 in phases else []):
                xts = sp_x.tile([128, 8 * 128], BF16, tag="xts")
                nc.sync.dma_start(
                    xts[:].rearrange("p (c s) -> p c s", c=8),
                    xT_r[:, :, i * 128 : (i + 1) * 128],
                )
                xts_v = xts[:].rearrange("p (c s) -> p c s", c=8)

                for wsb, kind in ((wq_sb, "q"), (wk_sb, "k"), (wv_sb, "v")):
                    ps = pp_proj.tile([128, GD], F32, tag="proj")
                    wv_ = wsb[:].rearrange("p (c d) -> p c d", c=8)
                    for c in range(8):
                        nc.tensor.matmul(
                            ps[:],
                            xts_v[:, c, :],
                            wv_[:, c, :],
                            start=(c == 0),
                            stop=(c == 7),
                        )
                    if kind == "v":
                        # evict V heads into vaug (cols 0-63 of each 65-block)
                        nc.vector.tensor_copy(
                            vaug_v[:, i, :, 0:64],
                            ps[:].rearrange("p (h d) -> p h d", h=GH),
                        )
                    else:
                        # RoPE: rot = P*cos + swap(P)*sin  (halves pre-deinterleaved)
                        pse = sp_t.tile([128, GD], BF16, tag="pse")
                        nc.vector.tensor_copy(pse[:], ps[:])
                        pse_sw = pse[:].rearrange("p (h x d) -> p h x d", h=GH, x=2)[
                            :, :, ::-1, :
                        ]
                        tmp1 = sp_t.tile([128, GD], BF16, tag="tmp1")
                        tmp2 = sp_t.tile([128, GD], BF16, tag="tmp2")
                        rot = sp_t.tile([128, GD], BF16, tag="rot" + kind)
                        nc.vector.tensor_tensor(
                            tmp1[:], pse[:], cos_v[:, i, :], op=MULT
                        )
                        nc.vector.tensor_tensor(
                            tmp2[:].rearrange("p (h x d) -> p h x d", h=GH, x=2),
                            pse_sw,
                            sin_v[:, i, :].rearrange("p (h x d) -> p h x d", h=GH, x=2),
                            op=MULT,
                        )
                        nc.vector.tensor_add(rot[:], tmp1[:], tmp2[:])
                        dst = qt_sb if kind == "q" else kt_sb
                        for hh in range(2):
                            pst = pp_tr.tile([128, 128], BF16, tag="tr")
                            nc.tensor.transpose(
                                pst[:], rot[:, hh * 128 : (hh + 1) * 128], idn_sb[:]
                            )
                            nc.vector.tensor_copy(
                                dst[hh][:, i * 128 : (i + 1) * 128], pst[:]
                            )

        # ---------------- Phase 2: attention ------------------------------
        with (
            tc.tile_pool(name="p2wt", bufs=4) as sp_wt,
            tc.tile_pool(name="p2a", bufs=3) as sp_a,
            tc.tile_pool(name="p2rc", bufs=2) as sp_rc,
            tc.tile_pool(name="p2an", bufs=3) as sp_an,
            tc.tile_pool(name="p2st", bufs=4) as sp_st,
            tc.tile_pool(name="p2s", bufs=2, space="PSUM") as pp_s,
            tc.tile_pool(name="p2pv", bufs=1, space="PSUM") as pp_pv,
            tc.tile_pool(name="p2t", bufs=1, space="PSUM") as pp_t,
            tc.tile_pool(name="p2bt", bufs=1, space="PSUM") as pp_bt,
        ):
            for hp in (range(2) if 2 in phases else []):
                for qc in range(NQC):
                    nk = 4 * qc + 4  # k-blocks (<= diagonal)
                    pv = [
                        pp_pv.tile([65, 512], F32, tag=f"pv{e}", name=f"pv{e}") for e in range(2)
                    ]
                    prev = None
                    for ki in range(nk):
                        j = ki - 4 * qc  # >=0 only for diagonal blocks
                        diag = j >= 0
                        off = j * 128 if diag else 0
                        n = 512 - off
                        ps_s = pp_s.tile([128, 1024], F32, tag="s")
                        wt = sp_wt.tile([128, 1024], BF16, tag="wt")
                        for e in range(2):  # head within pair
                            po = 64 * e
                            nc.tensor.matmul(
                                ps_s[:, e * 512 : e * 512 + n],
                                kt_sb[hp][po : po + 64, ki * 128 : (ki + 1) * 128],
                                qt_sb[hp][
                                    po : po + 64, qc * 512 + off : (qc + 1) * 512
                                ],
                                start=True,
                                stop=True,
                            )
                        ps_v = ps_s[:].rearrange("p (e q) -> p e q", e=2)
                        wt_v = wt[:].rearrange("p (e q) -> p e q", e=2)
                        nc.scalar.activation(
                            wt_v[:, :, 0:n], ps_v[:, :, 0:n], Exp, scale=0.125
                        )
                        if diag:
                            nc.vector.tensor_tensor(
                                wt_v[:, :, 0:128],
                                wt_v[:, :, 0:128],
                                tri_sb[:].unsqueeze(1).broadcast_to([128, 2, 128]),
                                op=MULT,
                            )
                        cur = [(ki, off, n, wt), (ki, off, n, wt)]
                        if prev is not None:
                            for e in range(2):
                                pki, poff, pn, pwt = prev[e]
                                nc.tensor.matmul(
                                    pv[e][0:65, poff:512],
                                    vaug_v[:, pki, 2 * hp + e, :],
                                    pwt[:, e * 512 : e * 512 + pn],
                                    start=(pki == 0),
                                    stop=False,
                                )
                        prev = cur
                    for e in range(2):
                        pki, poff, pn, pwt = prev[e]
                        nc.tensor.matmul(
                            pv[e][0:65, poff:512],
                            vaug_v[:, pki, 2 * hp + e, :],
                            pwt[:, e * 512 : e * 512 + pn],
                            start=(pki == 0),
                            stop=True,
                        )
                    for e in range(2):
                        h = 2 * hp + e
                        a = sp_a.tile([65, 512], F32, tag="a")
                        nc.vector.tensor_copy(a[:], pv[e][0:65, :])
                        ps_t = pp_t.tile([128, 260], F32, tag="t")
                        for j in range(4):
                            nc.tensor.transpose(
                                ps_t[:, j * 65 : (j + 1) * 65],
                                a[0:65, j * 128 : (j + 1) * 128],
                                idn32_sb[0:65, 0:65],
                            )
                        ps_t_v = ps_t[:].rearrange("p (j e) -> p j e", j=4)
                        rc = sp_rc.tile([128, 4], F32, tag="rc")
                        nc.vector.reciprocal(
                            rc[:].rearrange("p (j e) -> p j e", e=1),
                            ps_t_v[:, :, 64:65],
                        )
                        an = sp_an.tile([128, 256], BF16, tag="an")
                        for j in range(4):
                            nc.vector.tensor_scalar(
                                an[:, j * 64 : (j + 1) * 64],
                                ps_t_v[:, j, 0:64],
                                rc[:, j : j + 1],
                                None,
                                op0=MULT,
                            )
                        for j in range(4):
                            qb = qc * 4 + j
                            ps_bt = pp_bt.tile([128, 128], BF16, tag="bt")
                            nc.tensor.transpose(
                                ps_bt[0:64, :],
                                an[:, j * 64 : (j + 1) * 64],
                                idn_sb[:],
                            )
                            if e == 0:
                                nc.vector.tensor_copy(
                                    mt_v[0:64, qb, hp, :], ps_bt[0:64, :]
                                )
                            else:
                                stg = sp_st.tile([64, 128], BF16, tag="stg")
                                nc.vector.tensor_copy(stg[:], ps_bt[0:64, :])
                                nc.sync.dma_start(
                                    mt_v[64:128, qb, hp, :], stg[:]
                                )

        # ---------------- Phase 3: output projection ----------------------
        with (
            tc.tile_pool(name="p3o", bufs=3) as sp_o,
            tc.tile_pool(name="p3ps", bufs=2, space="PSUM") as pp_o,
        ):
            wo_v = wo_sb[:].rearrange("p (c d) -> p c d", c=2)
            for qb in (range(NB) if 3 in phases else []):
                ob = sp_o.tile([128, D_MODEL], F32, tag="ob")
                for oc in range(2):
                    ps_o = pp_o.tile([128, 512], F32, tag="o")
                    for hp in range(2):
                        nc.tensor.matmul(
                            ps_o[:],
                            mt_v[:, qb, hp, :],
                            wo_v[:, hp, oc * 512 : (oc + 1) * 512],
                            start=(hp == 0),
                            stop=(hp == 1),
                        )
                    nc.vector.tensor_copy(ob[:, oc * 512 : (oc + 1) * 512], ps_o[:])
                nc.sync.dma_start(out[qb * 128 : (qb + 1) * 128, :], ob[:])

    nc.compile()
    return nc


_NC = None


def _host_tables():
    inv_freq = 1.0 / (10000.0 ** (np.arange(0, HEAD_DIM, 2, dtype=np.float32) / HEAD_DIM))
    pos = np.arange(SEQ, dtype=np.float32)
    freq = pos[:, None] * inv_freq[None, :]  # [SEQ, 32]
    cos = np.cos(freq).astype(np.float32)
    sin = np.sin(freq).astype(np.float32)
    cos4 = np.tile(np.concatenate([cos, cos], axis=1), (1, GH))  # [SEQ, 256]
    sin4 = np.tile(np.concatenate([-sin, sin], axis=1), (1, GH))
    tri = np.triu(np.ones((128, 128), dtype=np.float32))  # keep k<=q
    idn = np.eye(128, dtype=np.float32)
    perm = np.concatenate(
        [h * 64 + np.concatenate([np.arange(0, 64, 2), np.arange(1, 64, 2)]) for h in range(GH)]
    )
    return cos4, sin4, tri, idn, perm


def _in_maps(x, qw, kw, vw, ow):
    cos4, sin4, tri, idn, perm = _host_tables()
    maps = []
    for c in range(N_CORES):
        b, g = c // GH, c % GH
        sl = slice(g * GD, (g + 1) * GD)
        maps.append(
            dict(
                xT=np.ascontiguousarray(x[b].T).astype(BF),
                wq=np.ascontiguousarray(qw[:, sl][:, perm]).astype(BF),
                wk=np.ascontiguousarray(kw[:, sl][:, perm]).astype(BF),
                wv=np.ascontiguousarray(vw[:, sl]).astype(BF),
                wo=np.ascontiguousarray(ow[sl, :]).astype(BF),
                ones=np.ones((128, NB * GH), dtype=BF),
                cos4=cos4.astype(BF),
                sin4=sin4.astype(BF),
                tri=tri.astype(BF),
                idn=idn.astype(BF),
                idn32=idn,
            )
        )
    return maps


def _run(x, qw, kw, vw, ow, trace=False):
    global _NC
    if _NC is None:
        _NC = build_program()
    maps = _in_maps(
        np.asarray(x, dtype=np.float32),
        np.asarray(qw, dtype=np.float32),
        np.asarray(kw, dtype=np.float32),
        np.asarray(vw, dtype=np.float32),
        np.asarray(ow, dtype=np.float32),
    )
    br = run_bass_kernel_spmd(_NC, maps, list(range(N_CORES)), trace=trace)
    out = np.zeros((BATCH, SEQ, D_MODEL), dtype=np.float32)
    for c in range(N_CORES):
        out[c // GH] += br.results[c]["out"]
    return out, br


def kernel(x, qw, kw, vw, ow):
    out, _ = _run(x, qw, kw, vw, ow)
    return out
